# revision 11
# baseline (speedup 1.0000x reference)
"""Trainium2 Bass kernel for nn_DestroyAgent (gnn_message_passing).

Math (algebraically reduced from the reference):
  h0 = coords @ W_embed + b_embed                      [N, 64]
  3 GNN layers: h = relu(h@Wself_l + segsum(h[src]@Wneigh_l, dst) + b_l) + h
  xg = MLP3(h3)[:, 0] per node;  z = MLP3(h0)[:, 0] per node
  out[b, k] = sum_n xg[b, n] * Wp[n] + sum_j z[b, ids[b,k,j]] * Wp[55+j] + bp

Sharding: 8 cores, core c owns graphs [32c, 32c+32) = nodes [1760c, 1760c+1760).
Layer 0: u0 = coordsAug @ (Waug@Wneigh0) is computed locally for ALL N (cheap,
no collective), then pull-mode segment sum over own-dst edges.
Layers 1-2: push mode — each core computes u = h@Wneigh for its OWN nodes,
gathers u[src] for its own-src edges (local 450KB buffer), scatter-accumulates
partial sums for ALL N dst nodes via one-hot matmuls into [8, 64, 1792] slabs,
then one ReduceScatter (458KB out) delivers each core its own nodes' agg.
This replaces the 3.6MB-out AllGather (~4x cheaper collective).
"""

import numpy as np
from contextlib import ExitStack

import concourse.bass as bass
import concourse.tile as tile
from concourse import bacc, mybir
from concourse.bass_utils import run_bass_kernel_spmd

dt = mybir.dt
F32 = dt.float32

B, K, D = 256, 256, 3
N_PER = 55
N = B * N_PER            # 14080
E = N * 8                # 112640
EMB = 64
NC_ = 8                  # cores
BPC = B // NC_           # graphs per core = 32
NPC = BPC * N_PER        # nodes per core = 1760
NPAD = 1792              # 14 * 128
NBLK = 14                # local 128-dst blocks
GBLK = NC_ * NBLK        # global dst blocks = 112
CHUNK = 448              # dense free-dim chunk (4 per core)
GCH = 8                  # gather chunks per layer

_cache = {}


def _tile_schedule(edge_lists, nblocks):
    """edge_lists[c][b] = (src_rows, dloc). Returns per-block tile counts
    (max over cores), total T, and per-core padded [T*128] src/dloc arrays."""
    ntb = [max(1, max((len(edge_lists[c][b][0]) + 127) // 128
                      for c in range(NC_)))
           for b in range(nblocks)]
    T = sum(ntb)
    srcs, dlocs = [], []
    for c in range(NC_):
        s_parts, d_parts = [], []
        for b in range(nblocks):
            es, ed = edge_lists[c][b]
            pad = ntb[b] * 128 - len(es)
            s_parts.append(np.concatenate([es, np.zeros(pad, np.int64)]))
            d_parts.append(np.concatenate([ed, np.full(pad, -1, np.int64)]))
        srcs.append(np.concatenate(s_parts))
        dlocs.append(np.concatenate(d_parts))
    src_all = np.stack(srcs)                     # [8, T*128]
    dloc_all = np.stack(dlocs)
    # dloc_sb [8, 128, T] fp32: edge t*128+p -> [p, t]
    dloc_sb = dloc_all.reshape(NC_, T, 128).transpose(0, 2, 1).astype(np.float32)
    # src idx wrap: idx i at [i%16, i//16], replicated x8 over partitions
    sw = src_all.reshape(NC_, T * 8, 16).transpose(0, 2, 1).astype(np.int16)
    src_wr = np.tile(sw, (1, 8, 1))              # [8, 128, T*8]
    return ntb, T, dloc_sb, src_wr


def _preprocess(coords, src, dst, destroy_ids):
    """Host-side index preprocessing -> per-core arrays + tile schedules."""
    src = np.asarray(src).astype(np.int64)
    dst = np.asarray(dst).astype(np.int64)

    # ---- pull schedule (layer 0): own-dst edges, 14 local blocks ----
    order = np.argsort(dst, kind="stable")
    src_s = src[order]
    dst_s = dst[order]
    pull = []
    for c in range(NC_):
        lo_n = c * NPC
        blocks = []
        for b in range(NBLK):
            d0 = lo_n + 128 * b
            d1 = min(lo_n + 128 * (b + 1), lo_n + NPC)
            lo = np.searchsorted(dst_s, d0)
            hi = np.searchsorted(dst_s, d1)
            e_src = src_s[lo:hi]
            e_dst = dst_s[lo:hi]
            o = np.argsort(e_src, kind="stable")   # src-sorted for DMA locality
            blocks.append((e_src[o], e_dst[o] - d0))
        pull.append(blocks)
    ntb, T, dloc_sb, src_wr = _tile_schedule(pull, NBLK)

    # ---- push schedule (layers 1-2): own-src edges, 112 global dst blocks ----
    own = src // NPC
    loc = dst % NPC
    gb_of = (dst // NPC) * NBLK + loc // 128
    dl_of = loc % 128
    push = []
    for c in range(NC_):
        m = own == c
        es = src[m] - c * NPC          # local row into ub
        gbs = gb_of[m]
        dls = dl_of[m]
        o = np.argsort(gbs, kind="stable")
        es, gbs, dls = es[o], gbs[o], dls[o]
        blocks = []
        for gb in range(GBLK):
            lo = np.searchsorted(gbs, gb)
            hi = np.searchsorted(gbs, gb + 1)
            o2 = np.argsort(es[lo:hi], kind="stable")
            blocks.append((es[lo:hi][o2], dls[lo:hi][o2]))
        push.append(blocks)
    ntbP, TP, dlocP_sb, srcP_wr = _tile_schedule(push, GBLK)

    # destroy ids per core: [128, BPC*2*3] fp32, col = (g*2 + h)*3 + j
    ids = destroy_ids.reshape(NC_, BPC, 2, 128, D)      # [c, g, h, p, j]
    idsf = ids.transpose(0, 3, 1, 2, 4).reshape(NC_, 128, BPC * 2 * D)
    idsf = idsf.astype(np.float32)

    return dict(ntb=ntb, T=T, dloc_sb=dloc_sb, src_wr=src_wr,
                ntbP=ntbP, TP=TP, dlocP_sb=dlocP_sb, srcP_wr=srcP_wr,
                idsf=idsf)


def _build(sched, reps=1, stage=99):
    ntb, T = sched["ntb"], sched["T"]
    ntbP, TP = sched["ntbP"], sched["TP"]
    key = (tuple(ntb), T, tuple(ntbP), TP, reps, stage)
    if key in _cache:
        return _cache[key]
    nc = bacc.Bacc("TRN2", target_bir_lowering=False, debug=False, num_devices=NC_)

    def din(name, shape, dtype=F32):
        return nc.dram_tensor(name, list(shape), dtype, kind="ExternalInput").ap()

    coordsT = din("coordsT", [3, NPAD])
    coordsF_d = din("coordsF", [3, N])
    w0p_d = din("w0p", [3, EMB])
    src_wr_d = din("src_wr", [128, T * 8], dt.int16)
    dloc_d = din("dloc", [128, T])
    srcP_wr_d = din("srcP_wr", [128, TP * 8], dt.int16)
    dlocP_d = din("dlocP", [128, TP])
    idsf_d = din("idsf", [128, BPC * 2 * D])
    iota128_d = din("iota128", [128, 128])
    iota55_d = din("iota55", [128, 55])
    ones128_d = din("ones128", [1, 128])
    waug_d = din("waug", [3, EMB])
    wself_d = din("wself", [EMB, 3, EMB])
    wneigh_d = din("wneigh", [EMB, 3, EMB])
    gnnbT_d = din("gnnbT", [EMB, 3])
    w1_d = din("w1", [EMB, 32])
    b1_d = din("b1", [32, 1])
    w2_d = din("w2", [32, 32])
    b2_d = din("b2", [32, 1])
    w3_d = din("w3", [32, 1])
    b3_d = din("b3", [1, 1])
    wpT55_d = din("wpT55", [1, 55])
    wp3_d = din("wp3", [1, 3])
    bp_d = din("bp", [1, 1])

    out_d = nc.dram_tensor("out", [BPC, K], F32, kind="ExternalOutput").ap()

    # DRAM scratch: u0 (all N, layer 0), per-layer local u rows, RS in/out
    u0s = [nc.dram_tensor(f"u0_{r}", [N, EMB], F32) for r in range(reps)]
    ubs = [[nc.dram_tensor(f"ub{r}_{l}", [NPAD, EMB], F32) for l in range(2)]
           for r in range(reps)]
    aggPs = [[nc.dram_tensor(f"aggP{r}_{l}", [NC_, EMB, NPAD], F32)
              for l in range(2)] for r in range(reps)]
    aggRs = [[nc.dram_tensor(f"aggR{r}_{l}", [EMB, NPAD], F32)
              for l in range(2)] for r in range(reps)]

    def chunks_of(Tn):
        tpc = (Tn + GCH - 1) // GCH
        szs = [min(tpc, Tn - g * tpc) for g in range(GCH) if Tn - g * tpc > 0]
        return tpc, szs

    TPC, CHSZ = chunks_of(T)
    TPCP, CHSZP = chunks_of(TP)

    def sched_maps(ntb_):
        block_of, first_t, last_t = [], {}, {}
        for b in range(len(ntb_)):
            block_of += [b] * ntb_[b]
        for t, b in enumerate(block_of):
            if b not in first_t:
                first_t[b] = t
            last_t[b] = t
        return block_of, first_t, last_t

    blk0, fst0, lst0 = sched_maps(ntb)
    blkP, fstP, lstP = sched_maps(ntbP)

    with tile.TileContext(nc) as tc, ExitStack() as ctx:
        sb = ctx.enter_context(tc.tile_pool(name="sb", bufs=1))
        sb2 = ctx.enter_context(tc.tile_pool(name="sb2", bufs=2))
        sb4 = ctx.enter_context(tc.tile_pool(name="sb4", bufs=4))
        sbg = ctx.enter_context(tc.tile_pool(name="sbg", bufs=2))

        def load(name, ap, shape, dtype=F32):
            t = sb.tile(list(shape), dtype, tag=name)
            nc.sync.dma_start(t[:], ap[:])
            return t

        coords_t = load("coords", coordsT, [3, NPAD])
        w0p = load("w0pt", w0p_d, [3, EMB])
        srcw_t = load("srcw", src_wr_d, [128, T * 8], dt.int16)
        dloc_t = load("dloct", dloc_d, [128, T])
        srcwP_t = load("srcwP", srcP_wr_d, [128, TP * 8], dt.int16)
        dlocP_t = load("dlocPt", dlocP_d, [128, TP])
        idsf_t = load("idsft", idsf_d, [128, BPC * 2 * D])
        iota128 = load("iota128t", iota128_d, [128, 128])
        iota55 = load("iota55t", iota55_d, [128, 55])
        ones128 = load("ones128t", ones128_d, [1, 128])
        waug = load("waugt", waug_d, [3, EMB])
        wself = load("wselft", wself_d, [EMB, 3, EMB])
        wneigh = load("wneight", wneigh_d, [EMB, 3, EMB])
        gnnbT = load("gnnbTt", gnnbT_d, [EMB, 3])
        w1 = load("w1t", w1_d, [EMB, 32])
        b1c = load("b1t", b1_d, [32, 1])
        w2 = load("w2t", w2_d, [32, 32])
        b2c = load("b2t", b2_d, [32, 1])
        w3 = load("w3t", w3_d, [32, 1])
        b3c = load("b3t", b3_d, [1, 1])
        wpT55 = load("wpT55t", wpT55_d, [1, 55])
        wp3 = load("wp3t", wp3_d, [1, 3])
        bp_t = load("bpt", bp_d, [1, 1])

        for r in range(reps):
            with tc.tile_pool(name=f"ps{r}", bufs=2, space="PSUM") as ps, \
                 tc.tile_pool(name=f"pb{r}", bufs=3, space="PSUM") as psb, \
                 tc.tile_pool(name=f"pz{r}", bufs=1, space="PSUM") as psz:
                # ---- h0 = Waug.T @ coordsT (bias via ones row) ----
                h0T = sb.tile([EMB, NPAD], F32, tag="h0T")
                for k in range(NPAD // CHUNK):
                    p = ps.tile([EMB, CHUNK], F32, tag="pre")
                    nc.tensor.matmul(p[:], waug[:], coords_t[:, bass.ts(k, CHUNK)],
                                     start=True, stop=True)
                    nc.vector.tensor_copy(h0T[:, bass.ts(k, CHUNK)], p[:])

                def mlp3(srcT, nm):
                    x1 = sb.tile([32, NPAD], F32, tag="x1s", name=f"x1_{nm}_{r}")
                    x2 = sb.tile([32, NPAD], F32, tag="x2s", name=f"x2_{nm}_{r}")
                    row = sb.tile([1, NPAD], F32, tag=f"row{nm}")
                    for k in range(NPAD // CHUNK):
                        sl = bass.ts(k, CHUNK)
                        p1 = ps.tile([32, CHUNK], F32, tag="u_ps")
                        nc.tensor.matmul(p1[:], w1[:], srcT[:, sl], start=True, stop=True)
                        nc.scalar.activation(x1[:, sl], p1[:],
                                             mybir.ActivationFunctionType.Lrelu,
                                             bias=b1c[:], alpha=0.01)
                        p2 = ps.tile([32, CHUNK], F32, tag="u_ps")
                        nc.tensor.matmul(p2[:], w2[:], x1[:, sl], start=True, stop=True)
                        nc.scalar.activation(x2[:, sl], p2[:],
                                             mybir.ActivationFunctionType.Lrelu,
                                             bias=b2c[:], alpha=0.01)
                        p3 = ps.tile([1, CHUNK], F32, tag="pre")
                        nc.tensor.matmul(p3[:], w3[:], x2[:, sl], start=True, stop=True)
                        nc.scalar.activation(row[:, sl], p3[:],
                                             mybir.ActivationFunctionType.Identity,
                                             bias=b3c[0:1, 0:1])
                    return row

                def gather_chunks(yb_ap, srcw, szs, tpc, nm):
                    gchunks = []
                    for g in range(len(szs)):
                        sz = szs[g]
                        gt = sbg.tile([128, sz, EMB], F32, tag="gath",
                                      name=f"gt{nm}_{g}")
                        nc.gpsimd.dma_gather(
                            gt[:], yb_ap,
                            srcw[:, g * tpc * 8:(g * tpc + sz) * 8],
                            num_idxs=sz * 128, num_idxs_reg=sz * 128,
                            elem_size=EMB, single_packet=(sz * 128 <= 1024))
                        gchunks.append(gt)
                    return gchunks

                def agg_matmuls(gchunks, szs, tpc, dloc_tile, blk, fst, lst,
                                nm, on_close):
                    """One-hot scatter matmuls, streamed per chunk. on_close(b,
                    psum_tile) is called when block b's accumulation closes."""
                    pb_map = {}
                    for g in range(len(szs)):
                        ohb = sbg.tile([128, szs[g], 128], F32, tag="ohbuf",
                                       name=f"ohb{nm}_{g}")
                        for k in range(szs[g]):
                            t = g * tpc + k
                            nc.vector.tensor_scalar(
                                ohb[:, k, :], iota128[:], dloc_tile[:, t:t + 1],
                                None, mybir.AluOpType.is_equal)
                        for k in range(szs[g]):
                            t = g * tpc + k
                            b = blk[t]
                            if fst[b] == t:
                                pb_map[b] = psb.tile([EMB, 128], F32, tag="blk",
                                                     name=f"pbm{nm}_{b}")
                            nc.tensor.matmul(pb_map[b][:], gchunks[g][:, k, :],
                                             ohb[:, k, :],
                                             start=(fst[b] == t),
                                             stop=(lst[b] == t))
                            if lst[b] == t:
                                on_close(b, pb_map.pop(b))

                def dense_update(l, hcur, aggT):
                    hnext = sb.tile([EMB, NPAD], F32, tag=f"hh{1 - (l % 2)}")
                    for k in range(NPAD // CHUNK):
                        sl = bass.ts(k, CHUNK)
                        p = ps.tile([EMB, CHUNK], F32, tag="pre")
                        nc.tensor.matmul(p[:], wself[:, l, :], hcur[:, sl],
                                         start=True, stop=True)
                        nc.vector.tensor_add(p[:], p[:], aggT[:, sl])
                        relu = sb2.tile([EMB, CHUNK], F32, tag="relu")
                        nc.scalar.activation(relu[:], p[:],
                                             mybir.ActivationFunctionType.Relu,
                                             bias=gnnbT[:, l:l + 1])
                        nc.vector.tensor_add(hnext[:, sl], relu[:], hcur[:, sl])
                    return hnext

                # ======== layer 0: pull mode, u0 for ALL N computed locally ====
                def emit_layer0(hcur):
                    yb = u0s[r]
                    NG = 10                      # groups of 11 row-tiles
                    for gi in range(NG):
                        cf = sb2.tile([3, 11 * 128], F32, tag="cf", name=f"cf{r}_{gi}")
                        nc.sync.dma_start(cf[:], coordsF_d[:, bass.ts(gi, 11 * 128)])
                        ust = sb2.tile([128, 11, EMB], F32, tag="ust", name=f"ust{r}_{gi}")
                        for t in range(11):
                            pu = ps.tile([128, EMB], F32, tag="u_ps")
                            nc.tensor.matmul(pu[:], cf[:, bass.ts(t, 128)],
                                             w0p[:], start=True, stop=True)
                            nc.scalar.copy(ust[:, t, :], pu[:])
                        nc.sync.dma_start(
                            yb.ap()[bass.ts(gi, 11 * 128)]
                              .rearrange("(t p) f -> p t f", p=128), ust[:])

                    gch = gather_chunks(yb.ap(), srcw_t, CHSZ, TPC, f"0_{r}")
                    aggT = sb2.tile([EMB, NPAD], F32, tag="aggT")

                    def close0(b, pt):
                        nc.scalar.copy(aggT[:, bass.ts(b, 128)], pt[:])
                    agg_matmuls(gch, CHSZ, TPC, dloc_t, blk0, fst0, lst0,
                                f"0_{r}", close0)
                    return dense_update(0, hcur, aggT)

                # ======== layers 1-2: push mode + ReduceScatter ========
                def emit_push(l, hcur, filler=None):
                    li = l - 1
                    ub, aggP, aggR = ubs[r][li], aggPs[r][li], aggRs[r][li]
                    # u = h @ Wneigh_l for own nodes -> ub [1792, 64]
                    u_sb = sb2.tile([128, NBLK, EMB], F32, tag="u_sb")
                    for t in range(NBLK):
                        pu = ps.tile([128, EMB], F32, tag="u_ps")
                        nc.tensor.matmul(pu[:], hcur[:, bass.ts(t, 128)],
                                         wneigh[:, l, :], start=True, stop=True)
                        nc.scalar.copy(u_sb[:, t, :], pu[:])
                    nc.sync.dma_start(
                        ub.ap().rearrange("(t p) f -> p t f", p=128), u_sb[:])

                    if stage < 2:        # skip gather+agg+RS entirely
                        aggT = sb2.tile([EMB, NPAD], F32, tag="aggT")
                        nc.vector.memset(aggT[:], 0.0)
                        if filler is not None:
                            filler()
                        return dense_update(l, hcur, aggT)

                    gch = gather_chunks(ub.ap(), srcwP_t, CHSZP, TPCP, f"{l}_{r}")

                    # per-owner agg slabs from a 2-deep pool (owners close in
                    # order; DMA of owner o overlaps fill of owner o+1)
                    slab_map = {}

                    def closeP(b, pt):
                        o, bl = b // NBLK, b % NBLK
                        if o not in slab_map:
                            slab_map[o] = sb2.tile([EMB, NPAD], F32, tag="slabT",
                                                   name=f"slab{r}_{l}_{o}")
                        nc.scalar.copy(slab_map[o][:, bass.ts(bl, 128)], pt[:])
                        if bl == NBLK - 1:
                            nc.sync.dma_start(aggP.ap()[o], slab_map.pop(o)[:])
                    agg_matmuls(gch, CHSZP, TPCP, dlocP_t, blkP, fstP, lstP,
                                f"{l}_{r}", closeP)

                    if stage >= 3:
                        nc.gpsimd.collective_compute(
                            "ReduceScatter", mybir.AluOpType.add,
                            replica_groups=[list(range(NC_))],
                            ins=[aggP.ap().opt()], outs=[aggR.ap().opt()])

                    if filler is not None:
                        filler()        # independent work to hide the RS

                    aggT = sb2.tile([EMB, NPAD], F32, tag="aggT")
                    if stage >= 3:
                        nc.sync.dma_start(aggT[:], aggR.ap()[:])
                    else:
                        nc.vector.memset(aggT[:], 0.0)
                    return dense_update(l, hcur, aggT)

                h1T = emit_layer0(h0T)

                # ======== tail part A: z = MLP3(h0), zw = z*Wp3 ========
                stg = sb.tile([128, BPC * 2], F32, tag="stage")
                zrow = mlp3(h0T, "z")
                zw_all = sb.tile([1, BPC * 3 * 55], F32, tag="zw")
                z_v = (zrow[0:1, 0:NPC].rearrange("o (g n) -> o g n", n=55)
                       .unsqueeze(2).broadcast_to([1, BPC, 3, 55]))
                wp3_v = (wp3[0:1, :].unsqueeze(1).unsqueeze(3)
                         .broadcast_to([1, BPC, 3, 55]))
                nc.vector.tensor_tensor(
                    zw_all[0:1, :].rearrange("o (g j n) -> o g j n", j=3, n=55),
                    z_v, wp3_v, mybir.AluOpType.mult)

                def stg_range(g0, g1):
                    def emit():
                        for g in range(g0, g1):
                            zw_ps = psz.tile([128, 165], F32, tag="zwps")
                            nc.tensor.matmul(zw_ps[:], ones128[:],
                                             zw_all[0:1, g * 165:(g + 1) * 165],
                                             start=True, stop=True)
                            for h in range(2):
                                col = (g * 2 + h) * 3
                                oh3 = sb4.tile([128, 3, 55], F32, tag="oh3")
                                for j in range(3):
                                    nc.vector.tensor_scalar(
                                        oh3[:, j, :], iota55[:],
                                        idsf_t[:, col + j:col + j + 1],
                                        None, mybir.AluOpType.is_equal)
                                scr = sb2.tile([128, 165], F32, tag="scr")
                                nc.vector.tensor_tensor(
                                    scr[:], oh3[:].rearrange("p a b -> p (a b)"),
                                    zw_ps[:], mybir.AluOpType.mult)
                                nc.vector.tensor_reduce(
                                    stg[:, g * 2 + h:g * 2 + h + 1],
                                    scr[:].rearrange("p (a b) -> p a b", b=165),
                                    axis=mybir.AxisListType.XY, op=mybir.AluOpType.add)
                    return emit

                h2T = emit_push(1, h1T, filler=stg_range(0, BPC // 2))
                h3T = emit_push(2, h2T, filler=stg_range(BPC // 2, BPC))

                # ======== tail part B: xg from h3 ========
                xgrow = mlp3(h3T, "xg")
                wpx = sb.tile([1, NPC], F32, tag="wpx")
                xg_v = xgrow[0:1, 0:NPC].rearrange("o (g n) -> o g n", n=55)
                wp_v = wpT55[0:1, :].unsqueeze(1).broadcast_to([1, BPC, 55])
                nc.vector.tensor_tensor(
                    wpx[0:1, :].rearrange("o (g n) -> o g n", n=55),
                    xg_v, wp_v, mybir.AluOpType.mult)
                s_row = sb.tile([1, BPC], F32, tag="srow")
                nc.vector.tensor_reduce(
                    s_row[:], wpx[0:1, :].rearrange("o (g n) -> o g n", n=55),
                    axis=mybir.AxisListType.X, op=mybir.AluOpType.add)
                nc.vector.tensor_scalar(s_row[:], s_row[:], bp_t[0:1, 0:1], None,
                                        mybir.AluOpType.add)
                sg_ps = ps.tile([128, BPC], F32, tag="pre")
                nc.tensor.matmul(sg_ps[:], ones128[:], s_row[:], start=True, stop=True)
                outb = sb.tile([128, BPC * 2], F32, tag="outb")
                nc.vector.tensor_tensor(
                    outb[:].rearrange("p (g h) -> p g h", h=2),
                    stg[:].rearrange("p (g h) -> p g h", h=2),
                    sg_ps[:].unsqueeze(2).broadcast_to([128, BPC, 2]),
                    mybir.AluOpType.add)
                nc.sync.dma_start(
                    out_d.rearrange("g (h p) -> p g h", p=128),
                    outb[:].rearrange("p (g h) -> p g h", h=2))

    nc.finalize()
    _cache[key] = nc
    return nc


def prepare(**inputs):
    coords = np.asarray(inputs["coords"], np.float32)
    src = np.asarray(inputs["src"])
    dst = np.asarray(inputs["dst"])
    destroy_ids = np.asarray(inputs["destroy_ids"])
    W_embed = np.asarray(inputs["W_embed"], np.float32)
    b_embed = np.asarray(inputs["b_embed"], np.float32)
    Wself = np.asarray(inputs["Wself"], np.float32)
    Wneigh = np.asarray(inputs["Wneigh"], np.float32)
    gnn_b = np.asarray(inputs["gnn_b"], np.float32)
    W1 = np.asarray(inputs["W1"], np.float32)
    b1 = np.asarray(inputs["b1"], np.float32)
    W2 = np.asarray(inputs["W2"], np.float32)
    b2 = np.asarray(inputs["b2"], np.float32)
    W3 = np.asarray(inputs["W3"], np.float32)
    b3 = np.asarray(inputs["b3"], np.float32)
    Wp = np.asarray(inputs["Wp"], np.float32)
    bp = np.asarray(inputs["bp"], np.float32)

    sched = _preprocess(coords, src, dst, destroy_ids)
    nc = _build(sched, reps=1)

    waug = np.concatenate([W_embed, b_embed[None, :]], 0)          # [3, 64]
    coordsF_full = np.concatenate([coords.T, np.ones((1, N), np.float32)], 0)
    w0p_np = (waug @ Wneigh[0]).astype(np.float32)
    iota128 = np.tile(np.arange(128, dtype=np.float32), (128, 1))
    iota55 = np.tile(np.arange(55, dtype=np.float32), (128, 1))
    ones128 = np.ones((1, 128), np.float32)

    in_maps = []
    for c in range(NC_):
        cs = coords[c * NPC:(c + 1) * NPC]                          # [1760, 2]
        coordsT = np.zeros((3, NPAD), np.float32)
        coordsT[:2, :NPC] = cs.T
        coordsT[2, :NPC] = 1.0
        in_maps.append({
            "coordsT": coordsT,
            "coordsF": coordsF_full, "w0p": w0p_np,
            "src_wr": sched["src_wr"][c],
            "dloc": sched["dloc_sb"][c],
            "srcP_wr": sched["srcP_wr"][c],
            "dlocP": sched["dlocP_sb"][c],
            "idsf": sched["idsf"][c],
            "iota128": iota128, "iota55": iota55, "ones128": ones128,
            "waug": waug,
            "wself": np.ascontiguousarray(Wself.transpose(1, 0, 2)),
            "wneigh": np.ascontiguousarray(Wneigh.transpose(1, 0, 2)),
            "gnnbT": gnn_b.T.copy(),
            "w1": W1, "b1": b1[:, None], "w2": W2, "b2": b2[:, None],
            "w3": W3, "b3": b3[:, None],
            "wpT55": Wp[:55, 0][None, :].copy(), "wp3": Wp[55:, 0][None, :].copy(),
            "bp": bp[:, None],
        })

    global _last_in_maps, _last_sched
    _last_in_maps = in_maps
    _last_sched = sched
    return nc, in_maps


def kernel(**inputs):
    nc, in_maps = prepare(**inputs)
    res = run_bass_kernel_spmd(nc, in_maps, core_ids=list(range(NC_)))
    return np.concatenate([res.results[c]["out"] for c in range(NC_)], 0)
